# revision 24
# baseline (speedup 1.0000x reference)
"""TRN2 Bass kernel for nn_BasicAttention (dense transformer attention block).

Full module: q/k/v projections -> per-head RMSNorm -> RoPE -> causal GQA
attention -> output projection.

Sharding: tensor-parallel over heads across 8 NeuronCores. Each core owns
2 query heads + 1 kv head (GQA group), computes attention for its heads,
and a partial output projection with its 256-row slice of Wo. The partials
are summed on the host (the unshard/all-reduce step).

v2: bf16 operands throughout (fp32 PSUM accumulation), paired score blocks
with one wide exp per pair, softmax denominator via DVE block-reduction +
a single ones-matmul per q-block (removes 288 PE row-sum matmuls),
software-pipelined output projection, coarse-grained DMA.

Self-contained: hardcodes all shapes; only needs /opt/trn_rl_repo (concourse)
on the python path, which is part of the environment.
"""
import sys

if "/opt/trn_rl_repo" not in sys.path:
    sys.path.insert(0, "/opt/trn_rl_repo")

import numpy as np

S = 4096       # sequence length
HID = 2048     # hidden size
H = 16         # query heads
HKV = 8        # kv heads
D = 128        # head dim
THETA = 10000.0
EPS = 1e-6
NCORES = 8
HPC = H // NCORES          # q heads per core = 2
MQKV = HPC * D + 2 * D     # projection cols per core: 256 q + 128 k + 128 v

_CACHE = {}


def _build(s_len):
    """Build the per-core Bass program (same program on all cores; inputs
    differ). Returns the compiled Bacc module."""
    import concourse.bacc as bacc
    import concourse.tile as tile
    from concourse import mybir

    f32 = mybir.dt.float32
    f32r = mybir.dt.float32r
    bf16 = mybir.dt.bfloat16

    n_sb = s_len // 512            # 512-wide seq blocks for projection phase
    n_kchunk = HID // 128          # 16 contraction chunks
    n_kb = s_len // 128            # attention k blocks
    n_qb = s_len // 512            # attention q blocks
    n_nb = HID // 512              # output hidden blocks

    nc = bacc.Bacc("TRN2", target_bir_lowering=False, debug=False)

    hiddenT = nc.dram_tensor("hiddenT", [HID, s_len], bf16, kind="ExternalInput").ap()
    wqkv = nc.dram_tensor("wqkv", [HID, MQKV], bf16, kind="ExternalInput").ap()
    wo = nc.dram_tensor("wo", [HPC * D, HID], bf16, kind="ExternalInput").ap()
    # norm weights etc, one column vector each
    qkw = nc.dram_tensor("qkw", [D, 4], f32, kind="ExternalInput").ap()
    # rope tables, stacked for the half-swap trick
    cosst = nc.dram_tensor("cosst", [D, s_len], bf16, kind="ExternalInput").ap()
    sinnst = nc.dram_tensor("sinnst", [D, s_len], bf16, kind="ExternalInput").ap()
    identc = nc.dram_tensor("identc", [128, 128], f32r, kind="ExternalInput").ap()
    onesc = nc.dram_tensor("onesc", [128, 128], bf16, kind="ExternalInput").ap()
    pswapc = nc.dram_tensor("pswapc", [128, 128], bf16, kind="ExternalInput").ap()
    out = nc.dram_tensor("out", [s_len, HID], bf16, kind="ExternalOutput").ap()

    with tile.TileContext(nc) as tc, \
         nc.allow_low_precision("bf16 attention: fp32 PSUM accumulation, "
                                "bf16 elementwise; verified vs fp64 reference"):
        with tc.tile_pool(name="const", bufs=1) as const, \
             tc.tile_pool(name="persist", bufs=1) as persist:
            ident_sb = const.tile([128, 128], f32r, name="ident_sb")
            ones_sb = const.tile([128, 128], bf16, name="ones_sb")
            pswap_sb = const.tile([128, 128], bf16, name="pswap_sb")
            qkw_sb = const.tile([128, 4], f32, name="qkw_sb")
            wo_sb = const.tile([128, HPC, HID], bf16, name="wo_sb")

            # preload the one ACT table set holding Ln+Exp+Copy so the
            # compiler's greedy per-function chooser never thrashes sets
            nc.scalar.add_instruction(mybir.InstLoadActFuncSet(
                name=nc.get_next_instruction_name(), act_func_set_id=6,
                ins=[], outs=[]))

            # persistent activations
            qkT = persist.tile([128, 3, s_len], bf16, name="qkT")  # qT h0, qT h1, kT
            v_sb = persist.tile([128, n_kb, 128], bf16, name="v_sb")

            # ---------------- Phase 1: projections + norm + rope ----------
            with tc.tile_pool(name="p1c", bufs=1) as p1c, \
                 tc.tile_pool(name="p1", bufs=2) as p1, \
                 tc.tile_pool(name="p1ps", bufs=1, space="PSUM") as p1ps, \
                 tc.tile_pool(name="ptps", bufs=2, space="PSUM") as ptps:
                csz = max(s_len // 4, 512)
                n_cch = s_len // csz
                cos_chunks = [
                    p1c.tile([128, csz], bf16, name=f"cosc{i}", tag=f"cosc{i}")
                    for i in range(n_cch)
                ]
                sinn_chunks = [
                    p1c.tile([128, csz], bf16, name=f"sinnc{i}", tag=f"sinnc{i}")
                    for i in range(n_cch)
                ]
                wqr = wqkv.rearrange("(k p) m -> p k m", p=128)
                hr = hiddenT.rearrange("(a p) s -> p a s", p=128)
                wq_quads = [
                    p1c.tile([128, 4, MQKV], bf16, name=f"wqq{i}", tag=f"wqq{i}")
                    for i in range(4)
                ]
                # first-needed data first: the first hidden quad, then wq
                # quad 0 split in 4 so the first matmul only waits on row 0,
                # then the remaining weight quads and hidden prefetches so
                # sb0 never runs dry; consts follow.
                hT4_pre = {}
                for kq in range(3):
                    t = p1.tile([128, 4, 512], bf16, name="hT4", tag="hT4",
                                bufs=3)
                    nc.sync.dma_start(t, hr[:, 4 * kq:4 * kq + 4, 0:512])
                    hT4_pre[kq] = t
                    if kq == 0:
                        for j in range(4):
                            nc.sync.dma_start(wq_quads[0][:, j, :],
                                              wqr[:, j, :])
                for q in range(1, 4):
                    nc.sync.dma_start(wq_quads[q], wqr[:, 4 * q:4 * q + 4, :])

                cptog = [0]
                deferred = []   # PE ops from the previous block's postprocess

                for sb in range(n_sb):
                    # 4 accumulating psum tiles, one per 128-col group of qkv
                    projps = [
                        p1ps.tile([128, 512], f32, name=f"projps{m}", tag=f"projps{m}")
                        for m in range(4)
                    ]
                    pend = []   # (k, hT4, kk) waiting for their proj matmuls

                    def flush_mm():
                        k0, hT0, kk0 = pend.pop(0)
                        for m in range(4):
                            nc.tensor.matmul(
                                projps[m],
                                wq_quads[k0 // 4][:, k0 % 4,
                                                  m * 128:(m + 1) * 128],
                                hT0[:, kk0, :],
                                start=(k0 == 0), stop=(k0 == n_kchunk - 1))
                        # interleave one deferred PE op from the previous
                        # block's postprocess; by now its inputs are ready
                        if deferred:
                            deferred.pop(0)()

                    for kq in range(4):
                        # one DMA brings 4 contraction chunks of hidden
                        if sb == 0 and kq in hT4_pre:
                            hT4 = hT4_pre[kq]
                        else:
                            hT4 = p1.tile([128, 4, 512], bf16, name="hT4",
                                          tag="hT4", bufs=3)
                            nc.sync.dma_start(
                                hT4, hr[:, 4 * kq:4 * kq + 4,
                                        sb * 512:(sb + 1) * 512])
                        if sb == 0 and kq == 0:
                            # consts needed from the first postprocess on;
                            # issued after the first wq/hidden loads
                            nc.sync.dma_start(pswap_sb, pswapc)
                            nc.sync.dma_start(ident_sb, identc)
                            nc.sync.dma_start(ones_sb, onesc)
                            nc.sync.dma_start(qkw_sb, qkw)
                        # rope-table chunks must be EMITTED before any rope
                        # op that reads them (emission order defines RAW vs
                        # WAR in Tile) -- chunks 0-2 land in sb0 kq1-3, the
                        # rest early in sb1 (first read is at sb6).
                        ci = None
                        if sb == 0 and 1 <= kq <= 3 and kq - 1 < n_cch:
                            ci = kq - 1
                        elif sb == 1 and kq + 3 < n_cch:
                            ci = kq + 3
                        if ci is not None:
                            nc.sync.dma_start(cos_chunks[ci],
                                              cosst[:, ci * csz:(ci + 1) * csz])
                            nc.sync.dma_start(sinn_chunks[ci],
                                              sinnst[:, ci * csz:(ci + 1) * csz])
                        for kk in range(4):
                            pend.append((kq * 4 + kk, hT4, kk))
                            if len(pend) >= 3:
                                flush_mm()
                    while pend:
                        flush_mm()

                    ssl = slice(sb * 512, (sb + 1) * 512)
                    # Free the psum banks fast: all copies + squares first.
                    # Everything downstream (stat matmuls, rope) is deferred
                    # into the next block's MM stream so PE never waits.
                    # The two q heads (m=0,1) share norm weight and score
                    # scale, so their stats run as one 1024-wide stream.
                    cpy01 = p1.tile([128, 2, 512], f32, name="cpy01",
                                    tag="cpy01", bufs=2)
                    nc.vector.tensor_copy(cpy01[:, 0, :], projps[0])
                    nc.vector.tensor_copy(cpy01[:, 1, :], projps[1])
                    cpy2 = p1.tile([128, 512], f32, name="cpy2", tag="cpy2",
                                   bufs=2)
                    nc.vector.tensor_copy(cpy2, projps[2])
                    # squares on GpSimd from the SBUF copies: keeps the psum
                    # free chain DVE-only and ACT out of the square work
                    sq01 = p1.tile([128, 2, 512], bf16, name="sq01",
                                   tag="sq01", bufs=2)
                    nc.gpsimd.tensor_mul(sq01, cpy01, cpy01)
                    sq2 = p1.tile([128, 512], bf16, name="sq2", tag="sq2",
                                  bufs=2)
                    nc.gpsimd.tensor_mul(sq2, cpy2, cpy2)
                    vT = p1.tile([128, 512], f32r, name="vT", tag="vT")
                    nc.vector.tensor_copy(vT, projps[3])

                    raws = {}

                    def emit_stats01(cpy01=cpy01, sq01=sq01):
                        ssps = p1ps.tile([128, 1024], f32, name="ssps",
                                         tag="ssps", bufs=1)
                        nc.tensor.matmul(ssps[:, 0:512], ones_sb,
                                         sq01[:, 0, :], start=True, stop=True)
                        nc.tensor.matmul(ssps[:, 512:1024], ones_sb,
                                         sq01[:, 1, :], start=True, stop=True)
                        tln = p1.tile([128, 1024], f32, name="tln", tag="tln")
                        nc.scalar.activation(
                            tln, ssps, mybir.ActivationFunctionType.Ln,
                            bias=qkw_sb[:, 2:3], scale=1.0 / 128.0)
                        rq = p1.tile([128, 1024], f32, name="rq", tag="rq")
                        # q heads fold the 1/sqrt(D) score scale in bias
                        nc.scalar.activation(
                            rq, tln, mybir.ActivationFunctionType.Exp,
                            bias=qkw_sb[:, 3:4], scale=-0.5)
                        raw01 = p1.tile([128, 2, 512], bf16, name="raw01",
                                        tag="raw01")
                        nc.vector.scalar_tensor_tensor(
                            raw01.rearrange("p a b -> p (a b)"),
                            cpy01.rearrange("p a b -> p (a b)"),
                            qkw_sb[:, 0:1], rq,
                            op0=mybir.AluOpType.mult,
                            op1=mybir.AluOpType.mult)
                        raws[0] = raw01[:, 0, :]
                        raws[1] = raw01[:, 1, :]

                    def emit_stats2(cpy2=cpy2, sq2=sq2):
                        # k head reuses half of the (bufs=1) wide psum tile
                        ssps = p1ps.tile([128, 1024], f32, name="ssps",
                                         tag="ssps", bufs=1)
                        nc.tensor.matmul(ssps[:, 0:512], ones_sb, sq2,
                                         start=True, stop=True)
                        tln = p1.tile([128, 512], f32, name="tln2", tag="tln2")
                        nc.scalar.activation(
                            tln, ssps[:, 0:512],
                            mybir.ActivationFunctionType.Ln,
                            bias=qkw_sb[:, 2:3], scale=1.0 / 128.0)
                        rq = p1.tile([128, 512], f32, name="rq2", tag="rq2")
                        nc.scalar.activation(
                            rq, tln, mybir.ActivationFunctionType.Exp,
                            scale=-0.5)
                        raw = p1.tile([128, 512], bf16, name="raw2",
                                      tag="raw2")
                        nc.vector.scalar_tensor_tensor(
                            raw, cpy2, qkw_sb[:, 1:2], rq,
                            op0=mybir.AluOpType.mult,
                            op1=mybir.AluOpType.mult)
                        raws[2] = raw

                    def make_rope(m, sb=sb):
                        def emit_rope():
                            raw = raws[m]
                            sslm = slice(sb * 512, (sb + 1) * 512)
                            # half-swap via PE permutation matmul
                            bsw = ptps.tile([128, 512], f32, name="bsw",
                                            tag="tps")
                            nc.tensor.matmul(bsw, pswap_sb, raw,
                                             start=True, stop=True)
                            ci, co = sb * 512 // csz, (sb * 512) % csz
                            ttc = p1.tile([128, 512], bf16, name="ttc",
                                          tag="ttc")
                            nc.vector.tensor_mul(
                                ttc, raw, cos_chunks[ci][:, co:co + 512])
                            tts = p1.tile([128, 512], bf16, name="tts",
                                          tag="tts")
                            nc.vector.tensor_mul(
                                tts, bsw, sinn_chunks[ci][:, co:co + 512])
                            nc.vector.tensor_add(qkT[:, m, sslm], ttc, tts)
                        return emit_rope

                    # rope0/rope1 before stats2: the DVE rope chain starts as
                    # soon as raw01 exists instead of queueing behind raw2's
                    # ACT dependency (matters at the phase boundary)
                    deferred.append(emit_stats01)
                    deferred.append(make_rope(0))
                    deferred.append(emit_stats2)
                    deferred.append(make_rope(1))
                    deferred.append(make_rope(2))

                    def emit_v(vT=vT, sb=sb):
                        vps = ptps.tile([128, 512], f32r, name="vps",
                                        tag="tps")
                        for j in range(4):
                            nc.tensor.transpose(
                                vps[:, j * 128:(j + 1) * 128],
                                vT[:, j * 128:(j + 1) * 128], ident_sb)
                        nc.vector.tensor_copy(
                            v_sb[:, 4 * sb:4 * sb + 4, :]
                            .rearrange("p a b -> p (a b)"),
                            vps)
                    deferred.append(emit_v)
                    if sb == n_sb - 1:
                        nc.sync.dma_start(
                            wo_sb, wo.rearrange("(h p) n -> p h n", p=128))
                while deferred:
                    deferred.pop(0)()

            # -------- Phases 2+3 interleaved: attention + output proj ------
            # Per (qb, h): kb pairs -> one [128,1024] psum, one wide exp into
            # a per-head slot buffer; PV matmuls accumulate per half.  The
            # softmax denominator: DVE adds the two halves of each pair
            # (contiguous reads), then an accumulating ones-matmul per pair
            # (144 total) does the partition reduction + broadcast in psum.
            # The denominator is ready at loop end, so each head's epilogue
            # follows its loop immediately; the output projection for qb-1
            # runs after both heads as pure filler PE work.
            with tc.tile_pool(name="p2s", bufs=2) as p2s, \
                 tc.tile_pool(name="oTp", bufs=4) as oTp, \
                 tc.tile_pool(name="p3", bufs=3) as p3, \
                 tc.tile_pool(name="ebp", bufs=1) as ebp, \
                 tc.tile_pool(name="scps_pool", bufs=2, space="PSUM") as scps_pool, \
                 tc.tile_pool(name="accps", bufs=1, space="PSUM") as accps, \
                 tc.tile_pool(name="mps", bufs=2, space="PSUM") as mps:
                ebufs = [
                    ebp.tile([128, n_qb * 2, 1024], bf16, name=f"ebuf{h}",
                             tag=f"ebuf{h}")
                    for h in range(HPC)
                ]
                cptog = [0]

                def attn_loop(qb, h, fillers):
                    qsl = slice(qb * 512, (qb + 1) * 512)
                    npair = 2 * qb + 2
                    ops = accps.tile([128, 512], f32, name="ops", tag="ops")
                    lps = accps.tile([128, 512], f32, name="lps", tag="lps")
                    esums = {}
                    for step in range(npair + 3):
                        if fillers:
                            fillers.pop(0)(True)
                        if step < npair:
                            p = step
                            kb0 = 2 * p
                            scps = scps_pool.tile([128, 1024], f32,
                                                  name="scps", tag="scps")
                            nc.tensor.matmul(
                                scps[:, 0:512],
                                qkT[:, 2, kb0 * 128:(kb0 + 1) * 128],
                                qkT[:, h, qsl], start=True, stop=True)
                            nc.tensor.matmul(
                                scps[:, 512:1024],
                                qkT[:, 2, (kb0 + 1) * 128:(kb0 + 2) * 128],
                                qkT[:, h, qsl], start=True, stop=True)
                            esb = ebufs[h][:, p, :]
                            nc.scalar.activation(
                                esb, scps, mybir.ActivationFunctionType.Exp)
                            if p >= 2 * qb:
                                # zero the k>q region of the diagonal pair
                                nc.gpsimd.affine_select(
                                    out=esb.rearrange("p (x q) -> p x q", x=2),
                                    in_=esb.rearrange("p (x q) -> p x q", x=2),
                                    compare_op=mybir.AluOpType.is_ge,
                                    fill=0.0,
                                    base=qb * 512 - kb0 * 128,
                                    pattern=[[-128, 2], [1, 512]],
                                    channel_multiplier=-1)
                        if step >= 1 and step - 1 < npair:
                            # pair-sum for the denominator (contiguous reads)
                            p = step - 1
                            esb = ebufs[h][:, p, :]
                            esum = p2s.tile([128, 512], bf16, name="esum",
                                            tag="esum", bufs=4)
                            nc.vector.tensor_add(esum, esb[:, 0:512],
                                                 esb[:, 512:1024])
                            esums[p] = esum
                        if step >= 3:
                            p = step - 3
                            kb0 = 2 * p
                            esb = ebufs[h][:, p, :]
                            nc.tensor.matmul(ops, v_sb[:, kb0, :],
                                             esb[:, 0:512],
                                             start=(p == 0), stop=False)
                            nc.tensor.matmul(ops, v_sb[:, kb0 + 1, :],
                                             esb[:, 512:1024],
                                             start=False, stop=(p == npair - 1))
                            nc.tensor.matmul(lps, ones_sb, esums.pop(p),
                                             start=(p == 0),
                                             stop=(p == npair - 1))
                    return ops, lps

                def emit_lfinish(ops, lps):
                    tl2 = p2s.tile([128, 512], f32, name="tl2", tag="tl2")
                    nc.scalar.activation(tl2, lps,
                                         mybir.ActivationFunctionType.Ln)
                    rl = p2s.tile([128, 512], f32, name="rl", tag="rl")
                    nc.scalar.activation(rl, tl2,
                                         mybir.ActivationFunctionType.Exp,
                                         scale=-1.0)
                    ot = oTp.tile([128, 512], bf16, name="ot", tag="ot")
                    nc.vector.tensor_mul(ot, ops, rl)
                    return ot

                def make_wo_units(qb, oTt):
                    # 16 single-(st4, nb) closures, interleaved one-per-step
                    # into the NEXT q-block's attention loops as PE filler
                    stg_state = {}

                    def make_unit(st4, nb):
                        def emit(in_loop):
                            st = qb * 4 + st4
                            stsl = slice(st * 128, (st + 1) * 128)
                            s4 = slice(st4 * 128, (st4 + 1) * 128)
                            if nb == 0:
                                stg_state[st4] = p3.tile(
                                    [128, n_nb, 512], bf16, name="stg4",
                                    tag="stg4")
                            stg4 = stg_state[st4]
                            nbsl = slice(nb * 512, (nb + 1) * 512)
                            wops = mps.tile([128, 512], f32, name="wops",
                                            tag="mps")
                            for h in range(HPC):
                                nc.tensor.matmul(wops, oTt[h][:, s4],
                                                 wo_sb[:, h, nbsl],
                                                 start=(h == 0),
                                                 stop=(h == HPC - 1))
                            # in-loop stages go to DVE (ACT paces the exp
                            # stream); flushed stages alternate ACT/DVE
                            if in_loop or cptog[0] % 2 == 0:
                                nc.vector.tensor_copy(stg4[:, nb, :], wops)
                            else:
                                nc.scalar.copy(stg4[:, nb, :], wops)
                            cptog[0] += 1
                            if nb == n_nb - 1:
                                nc.sync.dma_start(
                                    out[stsl, :],
                                    stg4.rearrange("p a b -> p (a b)"))
                        return emit

                    return [make_unit(st4, nb)
                            for st4 in range(4) for nb in range(n_nb)]

                # attention q-blocks in an order whose first block only needs
                # early-sequence K/V: the last seq block's rope chain then
                # overlaps the first attention loop instead of stalling PE
                fillers = []
                for qb in [2, 3, 4, 5, 6, 7, 0, 1]:
                    ops0, lps0 = attn_loop(qb, 0, fillers)
                    ot0 = emit_lfinish(ops0, lps0)
                    ops1, lps1 = attn_loop(qb, 1, fillers)
                    ot1 = emit_lfinish(ops1, lps1)
                    while fillers:
                        fillers.pop(0)(False)
                    fillers = make_wo_units(qb, [ot0, ot1])
                while fillers:
                    fillers.pop(0)(False)

    nc.compile()
    return nc


def _host_inputs(hidden_state, Wq, Wk, Wv, Wo, q_norm_w, k_norm_w, position_ids,
                 s_len):
    """Build the 8 per-core input maps."""
    import ml_dtypes
    bf16 = ml_dtypes.bfloat16

    half = D // 2
    pos = np.asarray(position_ids).astype(np.float64)
    inv_freq = 1.0 / (THETA ** (np.arange(half, dtype=np.float64) / half))
    ang = pos[:, None] * inv_freq[None, :]          # [S, half]
    cosT = np.cos(ang).T.astype(np.float32)         # [half, S]
    sinT = np.sin(ang).T.astype(np.float32)
    cosst = np.concatenate([cosT, cosT], axis=0).astype(bf16)       # [128, S]
    sinnst = np.concatenate([-sinT, sinT], axis=0).astype(bf16)     # [128, S]
    ident = np.eye(128, dtype=np.float32)
    ones = np.ones((128, 128), dtype=bf16)
    pswap = np.roll(np.eye(128), 64, axis=0).astype(bf16)
    hiddenT = np.asarray(hidden_state, dtype=np.float32).T.astype(bf16)
    qw = np.asarray(q_norm_w, dtype=np.float32)
    kw = np.asarray(k_norm_w, dtype=np.float32)
    epsc = np.full(D, EPS, dtype=np.float32)
    nbq = np.full(D, -0.5 * np.log(128.0), dtype=np.float32)
    qkw = np.stack([qw, kw, epsc, nbq], axis=1)     # [D, 4]

    in_maps = []
    for c in range(NCORES):
        wq_sl = Wq[:, c * HPC * D:(c + 1) * HPC * D]
        wk_sl = Wk[:, c * D:(c + 1) * D]
        wv_sl = Wv[:, c * D:(c + 1) * D]
        wqkv = np.concatenate([wq_sl, wk_sl, wv_sl], axis=1).astype(bf16)
        wo_sl = np.ascontiguousarray(
            Wo[c * HPC * D:(c + 1) * HPC * D, :]).astype(bf16)
        in_maps.append({
            "hiddenT": hiddenT,
            "wqkv": wqkv,
            "wo": wo_sl,
            "qkw": qkw,
            "cosst": cosst,
            "sinnst": sinnst,
            "identc": ident,
            "onesc": ones,
            "pswapc": pswap,
        })
    return in_maps


def kernel(hidden_state, Wq, Wk, Wv, Wo, q_norm_w, k_norm_w, position_ids,
           _s_len=None, _trace=False, **_ignored):
    from concourse.bass_utils import run_bass_kernel_spmd

    # accept jax or numpy inputs
    hidden_state = np.asarray(hidden_state)
    Wq, Wk, Wv, Wo = (np.asarray(w) for w in (Wq, Wk, Wv, Wo))
    q_norm_w = np.asarray(q_norm_w)
    k_norm_w = np.asarray(k_norm_w)
    position_ids = np.asarray(position_ids)

    s_len = int(hidden_state.shape[0]) if _s_len is None else _s_len
    if s_len not in _CACHE:
        _CACHE[s_len] = _build(s_len)
    nc = _CACHE[s_len]

    in_maps = _host_inputs(hidden_state, Wq, Wk, Wv, Wo, q_norm_w, k_norm_w,
                           position_ids, s_len)
    res = run_bass_kernel_spmd(nc, in_maps, core_ids=list(range(NCORES)),
                               trace=_trace)
    kernel._last = res
    acc = res.results[0]["out"].astype(np.float32)
    for c in range(1, NCORES):
        acc += res.results[c]["out"].astype(np.float32)
    return acc


# revision 30
# speedup vs baseline: 1.0133x; 1.0133x over previous
"""TRN2 Bass kernel for nn_BasicAttention (dense transformer attention block).

Full module: q/k/v projections -> per-head RMSNorm -> RoPE -> causal GQA
attention -> output projection.

Sharding: tensor-parallel over heads across 8 NeuronCores. Each core owns
2 query heads + 1 kv head (GQA group), computes attention for its heads,
and a partial output projection with its 256-row slice of Wo. The partials
are summed on the host (the unshard/all-reduce step).

v2: bf16 operands throughout (fp32 PSUM accumulation), paired score blocks
with one wide exp per pair, softmax denominator via DVE block-reduction +
a single ones-matmul per q-block (removes 288 PE row-sum matmuls),
software-pipelined output projection, coarse-grained DMA.

Self-contained: hardcodes all shapes; only needs /opt/trn_rl_repo (concourse)
on the python path, which is part of the environment.
"""
import sys

if "/opt/trn_rl_repo" not in sys.path:
    sys.path.insert(0, "/opt/trn_rl_repo")

import numpy as np

S = 4096       # sequence length
HID = 2048     # hidden size
H = 16         # query heads
HKV = 8        # kv heads
D = 128        # head dim
THETA = 10000.0
EPS = 1e-6
NCORES = 8
HPC = H // NCORES          # q heads per core = 2
MQKV = HPC * D + 2 * D     # projection cols per core: 256 q + 128 k + 128 v

_CACHE = {}


def _build(s_len):
    """Build the per-core Bass program (same program on all cores; inputs
    differ). Returns the compiled Bacc module."""
    import concourse.bacc as bacc
    import concourse.tile as tile
    from concourse import mybir

    f32 = mybir.dt.float32
    f32r = mybir.dt.float32r
    bf16 = mybir.dt.bfloat16

    n_sb = s_len // 512            # 512-wide seq blocks for projection phase
    n_kchunk = HID // 128          # 16 contraction chunks
    n_kb = s_len // 128            # attention k blocks
    n_qb = s_len // 512            # attention q blocks
    n_nb = HID // 512              # output hidden blocks

    nc = bacc.Bacc("TRN2", target_bir_lowering=False, debug=False)

    hiddenT = nc.dram_tensor("hiddenT", [HID, s_len], bf16, kind="ExternalInput").ap()
    wqkv = nc.dram_tensor("wqkv", [HID, MQKV], bf16, kind="ExternalInput").ap()
    wo = nc.dram_tensor("wo", [HPC * D, HID], bf16, kind="ExternalInput").ap()
    # norm weights etc, one column vector each
    qkw = nc.dram_tensor("qkw", [D, 4], f32, kind="ExternalInput").ap()
    # rope tables, stacked for the half-swap trick
    cosst = nc.dram_tensor("cosst", [D, s_len], bf16, kind="ExternalInput").ap()
    sinnst = nc.dram_tensor("sinnst", [D, s_len], bf16, kind="ExternalInput").ap()
    identc = nc.dram_tensor("identc", [128, 128], f32r, kind="ExternalInput").ap()
    onesc = nc.dram_tensor("onesc", [128, 128], bf16, kind="ExternalInput").ap()
    pswapc = nc.dram_tensor("pswapc", [128, 128], bf16, kind="ExternalInput").ap()
    out = nc.dram_tensor("out", [s_len, HID], bf16, kind="ExternalOutput").ap()

    with tile.TileContext(nc) as tc, \
         nc.allow_low_precision("bf16 attention: fp32 PSUM accumulation, "
                                "bf16 elementwise; verified vs fp64 reference"):
        with tc.tile_pool(name="const", bufs=1) as const, \
             tc.tile_pool(name="persist", bufs=1) as persist:
            ident_sb = const.tile([128, 128], f32r, name="ident_sb")
            ones_sb = const.tile([128, 128], bf16, name="ones_sb")
            pswap_sb = const.tile([128, 128], bf16, name="pswap_sb")
            qkw_sb = const.tile([128, 4], f32, name="qkw_sb")
            wo_sb = const.tile([128, HPC, HID], bf16, name="wo_sb")

            # preload the one ACT table set holding Ln+Exp+Copy so the
            # compiler's greedy per-function chooser never thrashes sets
            nc.scalar.add_instruction(mybir.InstLoadActFuncSet(
                name=nc.get_next_instruction_name(), act_func_set_id=6,
                ins=[], outs=[]))

            # persistent activations
            qkT = persist.tile([128, 3, s_len], bf16, name="qkT")  # qT h0, qT h1, kT
            v_sb = persist.tile([128, n_kb, 128], bf16, name="v_sb")

            # ---------------- Phase 1: projections + norm + rope ----------
            with tc.tile_pool(name="p1c", bufs=1) as p1c, \
                 tc.tile_pool(name="p1", bufs=2) as p1, \
                 tc.tile_pool(name="p1ps", bufs=1, space="PSUM") as p1ps, \
                 tc.tile_pool(name="ptps", bufs=2, space="PSUM") as ptps:
                csz = max(s_len // 4, 512)
                n_cch = s_len // csz
                cos_chunks = [
                    p1c.tile([128, csz], bf16, name=f"cosc{i}", tag=f"cosc{i}")
                    for i in range(n_cch)
                ]
                sinn_chunks = [
                    p1c.tile([128, csz], bf16, name=f"sinnc{i}", tag=f"sinnc{i}")
                    for i in range(n_cch)
                ]
                wqr = wqkv.rearrange("(k p) m -> p k m", p=128)
                hr = hiddenT.rearrange("(a p) s -> p a s", p=128)
                wq_quads = [
                    p1c.tile([128, 4, MQKV], bf16, name=f"wqq{i}", tag=f"wqq{i}")
                    for i in range(4)
                ]
                # first-needed data first: the first hidden quad, then wq
                # quad 0 split in 4 so the first matmul only waits on row 0,
                # then the remaining weight quads and hidden prefetches so
                # sb0 never runs dry; consts follow.
                hT4_pre = {}
                for kq in range(3):
                    t = p1.tile([128, 4, 512], bf16, name="hT4", tag="hT4",
                                bufs=3)
                    if kq == 0:
                        # first two chunks ASAP: the first matmul flush only
                        # needs chunks 0-2 + wq row 0
                        nc.sync.dma_start(t[:, 0:2, :], hr[:, 0:2, 0:512])
                        nc.sync.dma_start(wq_quads[0][:, 0, :], wqr[:, 0, :])
                        nc.sync.dma_start(t[:, 2:4, :], hr[:, 2:4, 0:512])
                        for j in range(1, 4):
                            nc.sync.dma_start(wq_quads[0][:, j, :],
                                              wqr[:, j, :])
                    else:
                        nc.sync.dma_start(t, hr[:, 4 * kq:4 * kq + 4, 0:512])
                    hT4_pre[kq] = t
                for q in range(1, 4):
                    nc.sync.dma_start(wq_quads[q], wqr[:, 4 * q:4 * q + 4, :])

                cptog = [0]
                deferred = []   # PE ops from the previous block's postprocess

                for sb in range(n_sb):
                    # 4 accumulating psum tiles, one per 128-col group of qkv
                    projps = [
                        p1ps.tile([128, 512], f32, name=f"projps{m}", tag=f"projps{m}")
                        for m in range(4)
                    ]
                    pend = []   # (k, hT4, kk) waiting for their proj matmuls

                    def flush_mm():
                        k0, hT0, kk0 = pend.pop(0)
                        for m in range(4):
                            nc.tensor.matmul(
                                projps[m],
                                wq_quads[k0 // 4][:, k0 % 4,
                                                  m * 128:(m + 1) * 128],
                                hT0[:, kk0, :],
                                start=(k0 == 0), stop=(k0 == n_kchunk - 1))
                        # interleave one deferred PE op from the previous
                        # block's postprocess; by now its inputs are ready
                        if deferred:
                            deferred.pop(0)()

                    for kq in range(4):
                        # one DMA brings 4 contraction chunks of hidden
                        if sb == 0 and kq in hT4_pre:
                            hT4 = hT4_pre[kq]
                        else:
                            hT4 = p1.tile([128, 4, 512], bf16, name="hT4",
                                          tag="hT4", bufs=3)
                            nc.sync.dma_start(
                                hT4, hr[:, 4 * kq:4 * kq + 4,
                                        sb * 512:(sb + 1) * 512])
                        if sb == 0 and kq == 0:
                            # consts needed from the first postprocess on;
                            # issued after the first wq/hidden loads
                            nc.sync.dma_start(pswap_sb, pswapc)
                            nc.sync.dma_start(ident_sb, identc)
                            nc.sync.dma_start(ones_sb, onesc)
                            nc.sync.dma_start(qkw_sb, qkw)
                        # rope-table chunks must be EMITTED before any rope
                        # op that reads them (emission order defines RAW vs
                        # WAR in Tile) -- chunks 0-2 land in sb0 kq1-3, the
                        # rest early in sb1 (first read is at sb6).
                        ci = None
                        if sb == 0 and 1 <= kq <= 3 and kq - 1 < n_cch:
                            ci = kq - 1
                        elif sb == 1 and kq + 3 < n_cch:
                            ci = kq + 3
                        if ci is not None:
                            nc.sync.dma_start(cos_chunks[ci],
                                              cosst[:, ci * csz:(ci + 1) * csz])
                            nc.sync.dma_start(sinn_chunks[ci],
                                              sinnst[:, ci * csz:(ci + 1) * csz])
                        for kk in range(4):
                            pend.append((kq * 4 + kk, hT4, kk))
                            if len(pend) >= 3:
                                flush_mm()
                    while pend:
                        flush_mm()

                    ssl = slice(sb * 512, (sb + 1) * 512)
                    # Free the psum banks fast: all copies + squares first.
                    # Everything downstream (stat matmuls, rope) is deferred
                    # into the next block's MM stream so PE never waits.
                    # The two q heads (m=0,1) share norm weight and score
                    # scale, so their stats run as one 1024-wide stream.
                    cpy01 = p1.tile([128, 2, 512], f32, name="cpy01",
                                    tag="cpy01", bufs=2)
                    nc.vector.tensor_copy(cpy01[:, 0, :], projps[0])
                    nc.vector.tensor_copy(cpy01[:, 1, :], projps[1])
                    cpy2 = p1.tile([128, 512], f32, name="cpy2", tag="cpy2",
                                   bufs=2)
                    nc.vector.tensor_copy(cpy2, projps[2])
                    # squares on GpSimd from the SBUF copies: keeps the psum
                    # free chain DVE-only and ACT out of the square work
                    sq01 = p1.tile([128, 2, 512], bf16, name="sq01",
                                   tag="sq01", bufs=2)
                    nc.gpsimd.tensor_mul(sq01, cpy01, cpy01)
                    sq2 = p1.tile([128, 512], bf16, name="sq2", tag="sq2",
                                  bufs=2)
                    nc.gpsimd.tensor_mul(sq2, cpy2, cpy2)
                    vT = p1.tile([128, 512], f32r, name="vT", tag="vT")
                    nc.vector.tensor_copy(vT, projps[3])

                    raws = {}

                    def emit_stats01(cpy01=cpy01, sq01=sq01):
                        ssps = p1ps.tile([128, 1024], f32, name="ssps",
                                         tag="ssps", bufs=1)
                        nc.tensor.matmul(ssps[:, 0:512], ones_sb,
                                         sq01[:, 0, :], start=True, stop=True)
                        nc.tensor.matmul(ssps[:, 512:1024], ones_sb,
                                         sq01[:, 1, :], start=True, stop=True)
                        tln = p1.tile([128, 1024], f32, name="tln", tag="tln")
                        nc.scalar.activation(
                            tln, ssps, mybir.ActivationFunctionType.Ln,
                            bias=qkw_sb[:, 2:3], scale=1.0 / 128.0)
                        rq = p1.tile([128, 1024], f32, name="rq", tag="rq")
                        # q heads fold the 1/sqrt(D) score scale in bias
                        nc.scalar.activation(
                            rq, tln, mybir.ActivationFunctionType.Exp,
                            bias=qkw_sb[:, 3:4], scale=-0.5)
                        raw01 = p1.tile([128, 2, 512], bf16, name="raw01",
                                        tag="raw01")
                        nc.vector.scalar_tensor_tensor(
                            raw01.rearrange("p a b -> p (a b)"),
                            cpy01.rearrange("p a b -> p (a b)"),
                            qkw_sb[:, 0:1], rq,
                            op0=mybir.AluOpType.mult,
                            op1=mybir.AluOpType.mult)
                        raws[0] = raw01[:, 0, :]
                        raws[1] = raw01[:, 1, :]

                    def emit_stats2(cpy2=cpy2, sq2=sq2):
                        # k head reuses half of the (bufs=1) wide psum tile
                        ssps = p1ps.tile([128, 1024], f32, name="ssps",
                                         tag="ssps", bufs=1)
                        nc.tensor.matmul(ssps[:, 0:512], ones_sb, sq2,
                                         start=True, stop=True)
                        tln = p1.tile([128, 512], f32, name="tln2", tag="tln2")
                        nc.scalar.activation(
                            tln, ssps[:, 0:512],
                            mybir.ActivationFunctionType.Ln,
                            bias=qkw_sb[:, 2:3], scale=1.0 / 128.0)
                        rq = p1.tile([128, 512], f32, name="rq2", tag="rq2")
                        nc.scalar.activation(
                            rq, tln, mybir.ActivationFunctionType.Exp,
                            scale=-0.5)
                        raw = p1.tile([128, 512], bf16, name="raw2",
                                      tag="raw2")
                        nc.vector.scalar_tensor_tensor(
                            raw, cpy2, qkw_sb[:, 1:2], rq,
                            op0=mybir.AluOpType.mult,
                            op1=mybir.AluOpType.mult)
                        raws[2] = raw

                    def make_rope(m, sb=sb):
                        def emit_rope():
                            raw = raws[m]
                            sslm = slice(sb * 512, (sb + 1) * 512)
                            # half-swap via PE permutation matmul
                            bsw = ptps.tile([128, 512], f32, name="bsw",
                                            tag="tps")
                            nc.tensor.matmul(bsw, pswap_sb, raw,
                                             start=True, stop=True)
                            ci, co = sb * 512 // csz, (sb * 512) % csz
                            # cos-mul on GpSimd (all-SBUF operands): runs in
                            # parallel with the DVE sin-mul, and keeps the
                            # phase-boundary DVE queue short
                            ttc = p1.tile([128, 512], bf16, name="ttc",
                                          tag="ttc")
                            nc.gpsimd.tensor_mul(
                                ttc, raw, cos_chunks[ci][:, co:co + 512])
                            tts = p1.tile([128, 512], bf16, name="tts",
                                          tag="tts")
                            nc.vector.tensor_mul(
                                tts, bsw, sinn_chunks[ci][:, co:co + 512])
                            nc.vector.tensor_add(qkT[:, m, sslm], ttc, tts)
                        return emit_rope

                    deferred.append(emit_stats01)
                    deferred.append(emit_stats2)
                    deferred.append(make_rope(0))
                    deferred.append(make_rope(1))
                    deferred.append(make_rope(2))

                    def emit_v(vT=vT, sb=sb):
                        vps = ptps.tile([128, 512], f32r, name="vps",
                                        tag="tps")
                        for j in range(4):
                            nc.tensor.transpose(
                                vps[:, j * 128:(j + 1) * 128],
                                vT[:, j * 128:(j + 1) * 128], ident_sb)
                        nc.vector.tensor_copy(
                            v_sb[:, 4 * sb:4 * sb + 4, :]
                            .rearrange("p a b -> p (a b)"),
                            vps)
                    deferred.append(emit_v)
                    if sb == n_sb - 1:
                        nc.sync.dma_start(
                            wo_sb, wo.rearrange("(h p) n -> p h n", p=128))
                def emit_warm(n):
                    # keep PE busy while the last block's stats/rope chains
                    # drain: a PE-idle window >3.4us here HAM-throttles the
                    # clock to 1.2GHz right as attention starts. Must be
                    # emitted BEFORE the waiting rope matmuls (PE is FIFO).
                    # Results are never read.
                    for _ in range(n):
                        wtile = ptps.tile([128, 512], f32, name="warm",
                                          tag="tps")
                        nc.tensor.matmul(wtile, ones_sb, qkT[:, 2, 0:512],
                                         start=True, stop=True)

                # final drain: the two stats closures launch their ACT
                # chains, then a warmup block covers the ~5us until raw01/
                # raw2 exist, then the rope/v closures drain.
                deferred.pop(0)()
                deferred.pop(0)()
                emit_warm(20)
                while deferred:
                    deferred.pop(0)()
                    emit_warm(3)

            # -------- Phases 2+3 interleaved: attention + output proj ------
            # Per (qb, h): kb pairs -> one [128,1024] psum, one wide exp into
            # a per-head slot buffer; PV matmuls accumulate per half.  The
            # softmax denominator: DVE adds the two halves of each pair
            # (contiguous reads), then an accumulating ones-matmul per pair
            # (144 total) does the partition reduction + broadcast in psum.
            # The denominator is ready at loop end, so each head's epilogue
            # follows its loop immediately; the output projection for qb-1
            # runs after both heads as pure filler PE work.
            with tc.tile_pool(name="p2s", bufs=2) as p2s, \
                 tc.tile_pool(name="oTp", bufs=4) as oTp, \
                 tc.tile_pool(name="p3", bufs=3) as p3, \
                 tc.tile_pool(name="ebp", bufs=1) as ebp, \
                 tc.tile_pool(name="scps_pool", bufs=2, space="PSUM") as scps_pool, \
                 tc.tile_pool(name="accps", bufs=1, space="PSUM") as accps, \
                 tc.tile_pool(name="mps", bufs=2, space="PSUM") as mps:
                ebufs = [
                    ebp.tile([128, n_qb * 2, 1024], bf16, name=f"ebuf{h}",
                             tag=f"ebuf{h}")
                    for h in range(HPC)
                ]
                cptog = [0]

                def attn_loop(qb, h, fillers):
                    qsl = slice(qb * 512, (qb + 1) * 512)
                    npair = 2 * qb + 2
                    ops = accps.tile([128, 512], f32, name="ops", tag="ops")
                    lps = accps.tile([128, 512], f32, name="lps", tag="lps")
                    esums = {}
                    for step in range(npair + 3):
                        if fillers:
                            fillers.pop(0)(True)
                        if step < npair:
                            p = step
                            kb0 = 2 * p
                            scps = scps_pool.tile([128, 1024], f32,
                                                  name="scps", tag="scps")
                            nc.tensor.matmul(
                                scps[:, 0:512],
                                qkT[:, 2, kb0 * 128:(kb0 + 1) * 128],
                                qkT[:, h, qsl], start=True, stop=True)
                            nc.tensor.matmul(
                                scps[:, 512:1024],
                                qkT[:, 2, (kb0 + 1) * 128:(kb0 + 2) * 128],
                                qkT[:, h, qsl], start=True, stop=True)
                            esb = ebufs[h][:, p, :]
                            nc.scalar.activation(
                                esb, scps, mybir.ActivationFunctionType.Exp)
                            if p >= 2 * qb:
                                # zero the k>q region of the diagonal pair
                                nc.gpsimd.affine_select(
                                    out=esb.rearrange("p (x q) -> p x q", x=2),
                                    in_=esb.rearrange("p (x q) -> p x q", x=2),
                                    compare_op=mybir.AluOpType.is_ge,
                                    fill=0.0,
                                    base=qb * 512 - kb0 * 128,
                                    pattern=[[-128, 2], [1, 512]],
                                    channel_multiplier=-1)
                        if step >= 1 and step - 1 < npair:
                            # pair-sum for the denominator (contiguous reads)
                            p = step - 1
                            esb = ebufs[h][:, p, :]
                            esum = p2s.tile([128, 512], bf16, name="esum",
                                            tag="esum", bufs=4)
                            nc.vector.tensor_add(esum, esb[:, 0:512],
                                                 esb[:, 512:1024])
                            esums[p] = esum
                        if step >= 3:
                            p = step - 3
                            kb0 = 2 * p
                            esb = ebufs[h][:, p, :]
                            nc.tensor.matmul(ops, v_sb[:, kb0, :],
                                             esb[:, 0:512],
                                             start=(p == 0), stop=False)
                            nc.tensor.matmul(ops, v_sb[:, kb0 + 1, :],
                                             esb[:, 512:1024],
                                             start=False, stop=(p == npair - 1))
                            nc.tensor.matmul(lps, ones_sb, esums.pop(p),
                                             start=(p == 0),
                                             stop=(p == npair - 1))
                    return ops, lps

                def emit_lfinish(ops, lps):
                    tl2 = p2s.tile([128, 512], f32, name="tl2", tag="tl2")
                    nc.scalar.activation(tl2, lps,
                                         mybir.ActivationFunctionType.Ln)
                    rl = p2s.tile([128, 512], f32, name="rl", tag="rl")
                    nc.scalar.activation(rl, tl2,
                                         mybir.ActivationFunctionType.Exp,
                                         scale=-1.0)
                    ot = oTp.tile([128, 512], bf16, name="ot", tag="ot")
                    nc.vector.tensor_mul(ot, ops, rl)
                    return ot

                def make_wo_units(qb, oTt):
                    # 16 single-(st4, nb) closures, interleaved one-per-step
                    # into the NEXT q-block's attention loops as PE filler
                    stg_state = {}

                    def make_unit(st4, nb):
                        def emit(in_loop):
                            st = qb * 4 + st4
                            stsl = slice(st * 128, (st + 1) * 128)
                            s4 = slice(st4 * 128, (st4 + 1) * 128)
                            if nb == 0:
                                stg_state[st4] = p3.tile(
                                    [128, n_nb, 512], bf16, name="stg4",
                                    tag="stg4")
                            stg4 = stg_state[st4]
                            nbsl = slice(nb * 512, (nb + 1) * 512)
                            if in_loop:
                                wops = mps.tile([128, 512], f32, name="wops",
                                                tag="mps")
                            else:
                                # flushed units rotate over 4 psum banks
                                # (mps x2 + the attention accumulators, idle
                                # during a flush) so stage copies overlap MMs
                                wtag = ("mps", "ops", "mps", "lps")[
                                    cptog[0] % 4]
                                wpool = mps if wtag == "mps" else accps
                                wops = wpool.tile([128, 512], f32,
                                                  name="wops", tag=wtag)
                            for h in range(HPC):
                                nc.tensor.matmul(wops, oTt[h][:, s4],
                                                 wo_sb[:, h, nbsl],
                                                 start=(h == 0),
                                                 stop=(h == HPC - 1))
                            # in-loop stages go to DVE (ACT paces the exp
                            # stream); flushed stages alternate ACT/DVE
                            if in_loop or cptog[0] % 2 == 0:
                                nc.vector.tensor_copy(stg4[:, nb, :], wops)
                            else:
                                nc.scalar.copy(stg4[:, nb, :], wops)
                            cptog[0] += 1
                            if nb == n_nb - 1:
                                nc.sync.dma_start(
                                    out[stsl, :],
                                    stg4.rearrange("p a b -> p (a b)"))
                        return emit

                    return [make_unit(st4, nb)
                            for st4 in range(4) for nb in range(n_nb)]

                # attention q-blocks in an order whose first block only needs
                # early-sequence K/V: the last seq block's rope chain then
                # overlaps the first attention loop instead of stalling PE
                fillers = []
                for qb in [2, 3, 4, 5, 6, 7, 0, 1]:
                    ops0, lps0 = attn_loop(qb, 0, fillers)
                    ot0 = emit_lfinish(ops0, lps0)
                    ops1, lps1 = attn_loop(qb, 1, fillers)
                    ot1 = emit_lfinish(ops1, lps1)
                    while fillers:
                        fillers.pop(0)(False)
                    fillers = make_wo_units(qb, [ot0, ot1])
                while fillers:
                    fillers.pop(0)(False)

    nc.compile()
    return nc


def _host_inputs(hidden_state, Wq, Wk, Wv, Wo, q_norm_w, k_norm_w, position_ids,
                 s_len):
    """Build the 8 per-core input maps."""
    import ml_dtypes
    bf16 = ml_dtypes.bfloat16

    half = D // 2
    pos = np.asarray(position_ids).astype(np.float64)
    inv_freq = 1.0 / (THETA ** (np.arange(half, dtype=np.float64) / half))
    ang = pos[:, None] * inv_freq[None, :]          # [S, half]
    cosT = np.cos(ang).T.astype(np.float32)         # [half, S]
    sinT = np.sin(ang).T.astype(np.float32)
    cosst = np.concatenate([cosT, cosT], axis=0).astype(bf16)       # [128, S]
    sinnst = np.concatenate([-sinT, sinT], axis=0).astype(bf16)     # [128, S]
    ident = np.eye(128, dtype=np.float32)
    ones = np.ones((128, 128), dtype=bf16)
    pswap = np.roll(np.eye(128), 64, axis=0).astype(bf16)
    hiddenT = np.asarray(hidden_state, dtype=np.float32).T.astype(bf16)
    qw = np.asarray(q_norm_w, dtype=np.float32)
    kw = np.asarray(k_norm_w, dtype=np.float32)
    epsc = np.full(D, EPS, dtype=np.float32)
    nbq = np.full(D, -0.5 * np.log(128.0), dtype=np.float32)
    qkw = np.stack([qw, kw, epsc, nbq], axis=1)     # [D, 4]

    in_maps = []
    for c in range(NCORES):
        wq_sl = Wq[:, c * HPC * D:(c + 1) * HPC * D]
        wk_sl = Wk[:, c * D:(c + 1) * D]
        wv_sl = Wv[:, c * D:(c + 1) * D]
        wqkv = np.concatenate([wq_sl, wk_sl, wv_sl], axis=1).astype(bf16)
        wo_sl = np.ascontiguousarray(
            Wo[c * HPC * D:(c + 1) * HPC * D, :]).astype(bf16)
        in_maps.append({
            "hiddenT": hiddenT,
            "wqkv": wqkv,
            "wo": wo_sl,
            "qkw": qkw,
            "cosst": cosst,
            "sinnst": sinnst,
            "identc": ident,
            "onesc": ones,
            "pswapc": pswap,
        })
    return in_maps


def kernel(hidden_state, Wq, Wk, Wv, Wo, q_norm_w, k_norm_w, position_ids,
           _s_len=None, _trace=False, **_ignored):
    from concourse.bass_utils import run_bass_kernel_spmd

    # accept jax or numpy inputs
    hidden_state = np.asarray(hidden_state)
    Wq, Wk, Wv, Wo = (np.asarray(w) for w in (Wq, Wk, Wv, Wo))
    q_norm_w = np.asarray(q_norm_w)
    k_norm_w = np.asarray(k_norm_w)
    position_ids = np.asarray(position_ids)

    s_len = int(hidden_state.shape[0]) if _s_len is None else _s_len
    if s_len not in _CACHE:
        _CACHE[s_len] = _build(s_len)
    nc = _CACHE[s_len]

    in_maps = _host_inputs(hidden_state, Wq, Wk, Wv, Wo, q_norm_w, k_norm_w,
                           position_ids, s_len)
    res = run_bass_kernel_spmd(nc, in_maps, core_ids=list(range(NCORES)),
                               trace=_trace)
    kernel._last = res
    acc = res.results[0]["out"].astype(np.float32)
    for c in range(1, NCORES):
        acc += res.results[c]["out"].astype(np.float32)
    return acc


# revision 38
# speedup vs baseline: 1.0182x; 1.0048x over previous
"""TRN2 Bass kernel for nn_BasicAttention (dense transformer attention block).

Full module: q/k/v projections -> per-head RMSNorm -> RoPE -> causal GQA
attention -> output projection.

Sharding: tensor-parallel over heads across 8 NeuronCores. Each core owns
2 query heads + 1 kv head (GQA group), computes attention for its heads,
and a partial output projection with its 256-row slice of Wo. The partials
are summed on the host (the unshard/all-reduce step).

v2: bf16 operands throughout (fp32 PSUM accumulation), paired score blocks
with one wide exp per pair, softmax denominator via DVE block-reduction +
a single ones-matmul per q-block (removes 288 PE row-sum matmuls),
software-pipelined output projection, coarse-grained DMA.

Self-contained: hardcodes all shapes; only needs /opt/trn_rl_repo (concourse)
on the python path, which is part of the environment.
"""
import sys

if "/opt/trn_rl_repo" not in sys.path:
    sys.path.insert(0, "/opt/trn_rl_repo")

import numpy as np

S = 4096       # sequence length
HID = 2048     # hidden size
H = 16         # query heads
HKV = 8        # kv heads
D = 128        # head dim
THETA = 10000.0
EPS = 1e-6
NCORES = 8
HPC = H // NCORES          # q heads per core = 2
MQKV = HPC * D + 2 * D     # projection cols per core: 256 q + 128 k + 128 v

_CACHE = {}


def _build(s_len):
    """Build the per-core Bass program (same program on all cores; inputs
    differ). Returns the compiled Bacc module."""
    import concourse.bacc as bacc
    import concourse.tile as tile
    from concourse import mybir

    f32 = mybir.dt.float32
    f32r = mybir.dt.float32r
    bf16 = mybir.dt.bfloat16

    n_sb = s_len // 512            # 512-wide seq blocks for projection phase
    n_kchunk = HID // 128          # 16 contraction chunks
    n_kb = s_len // 128            # attention k blocks
    n_qb = s_len // 512            # attention q blocks
    n_nb = HID // 512              # output hidden blocks

    nc = bacc.Bacc("TRN2", target_bir_lowering=False, debug=False)

    hiddenT = nc.dram_tensor("hiddenT", [HID, s_len], bf16, kind="ExternalInput").ap()
    wqkv = nc.dram_tensor("wqkv", [HID, MQKV], bf16, kind="ExternalInput").ap()
    wo = nc.dram_tensor("wo", [HPC * D, HID], bf16, kind="ExternalInput").ap()
    # norm weights etc, one column vector each
    qkw = nc.dram_tensor("qkw", [D, 4], f32, kind="ExternalInput").ap()
    # rope tables, stacked for the half-swap trick
    cosst = nc.dram_tensor("cosst", [D, s_len], bf16, kind="ExternalInput").ap()
    sinnst = nc.dram_tensor("sinnst", [D, s_len], bf16, kind="ExternalInput").ap()
    identc = nc.dram_tensor("identc", [128, 128], f32r, kind="ExternalInput").ap()
    onesc = nc.dram_tensor("onesc", [128, 128], bf16, kind="ExternalInput").ap()
    pswapc = nc.dram_tensor("pswapc", [128, 128], bf16, kind="ExternalInput").ap()
    out = nc.dram_tensor("out", [s_len, HID], bf16, kind="ExternalOutput").ap()

    with tile.TileContext(nc) as tc, \
         nc.allow_low_precision("bf16 attention: fp32 PSUM accumulation, "
                                "bf16 elementwise; verified vs fp64 reference"):
        with tc.tile_pool(name="const", bufs=1) as const, \
             tc.tile_pool(name="persist", bufs=1) as persist:
            ident_sb = const.tile([128, 128], f32r, name="ident_sb")
            ones_sb = const.tile([128, 128], bf16, name="ones_sb")
            pswap_sb = const.tile([128, 128], bf16, name="pswap_sb")
            qkw_sb = const.tile([128, 4], f32, name="qkw_sb")
            wo_sb = const.tile([128, HPC, HID], bf16, name="wo_sb")

            # preload the one ACT table set holding Ln+Exp+Copy so the
            # compiler's greedy per-function chooser never thrashes sets
            nc.scalar.add_instruction(mybir.InstLoadActFuncSet(
                name=nc.get_next_instruction_name(), act_func_set_id=6,
                ins=[], outs=[]))

            # persistent activations
            qkT = persist.tile([128, 3, s_len], bf16, name="qkT")  # qT h0, qT h1, kT
            v_sb = persist.tile([128, n_kb, 128], bf16, name="v_sb")

            # ---------------- Phase 1: projections + norm + rope ----------
            with tc.tile_pool(name="p1c", bufs=1) as p1c, \
                 tc.tile_pool(name="p1", bufs=2) as p1, \
                 tc.tile_pool(name="p1ps", bufs=1, space="PSUM") as p1ps, \
                 tc.tile_pool(name="ptps", bufs=2, space="PSUM") as ptps:
                csz = max(s_len // 4, 512)
                n_cch = s_len // csz
                cos_chunks = [
                    p1c.tile([128, csz], bf16, name=f"cosc{i}", tag=f"cosc{i}")
                    for i in range(n_cch)
                ]
                sinn_chunks = [
                    p1c.tile([128, csz], bf16, name=f"sinnc{i}", tag=f"sinnc{i}")
                    for i in range(n_cch)
                ]
                wqr = wqkv.rearrange("(k p) m -> p k m", p=128)
                hr = hiddenT.rearrange("(a p) s -> p a s", p=128)
                wq_quads = [
                    p1c.tile([128, 4, MQKV], bf16, name=f"wqq{i}", tag=f"wqq{i}")
                    for i in range(4)
                ]
                # first-needed data first: the first hidden quad, then wq
                # quad 0 split in 4 so the first matmul only waits on row 0,
                # then the remaining weight quads and hidden prefetches so
                # sb0 never runs dry; consts follow.
                hT4_pre = {}
                for kq in range(3):
                    t = p1.tile([128, 4, 512], bf16, name="hT4", tag="hT4",
                                bufs=3)
                    if kq == 0:
                        # first two chunks ASAP: the first matmul flush only
                        # needs chunks 0-2 + wq row 0
                        nc.sync.dma_start(t[:, 0:2, :], hr[:, 0:2, 0:512])
                        nc.sync.dma_start(wq_quads[0][:, 0, :], wqr[:, 0, :])
                        nc.sync.dma_start(t[:, 2:4, :], hr[:, 2:4, 0:512])
                        for j in range(1, 4):
                            nc.sync.dma_start(wq_quads[0][:, j, :],
                                              wqr[:, j, :])
                    else:
                        nc.sync.dma_start(t, hr[:, 4 * kq:4 * kq + 4, 0:512])
                    hT4_pre[kq] = t
                for q in range(1, 4):
                    nc.sync.dma_start(wq_quads[q], wqr[:, 4 * q:4 * q + 4, :])

                cptog = [0]
                deferred = []   # PE ops from the previous block's postprocess

                for sb in range(n_sb):
                    # 4 accumulating psum tiles, one per 128-col group of qkv
                    projps = [
                        p1ps.tile([128, 512], f32, name=f"projps{m}", tag=f"projps{m}")
                        for m in range(4)
                    ]
                    pend = []   # (k, hT4, kk) waiting for their proj matmuls

                    def flush_mm():
                        k0, hT0, kk0 = pend.pop(0)
                        for m in range(4):
                            nc.tensor.matmul(
                                projps[m],
                                wq_quads[k0 // 4][:, k0 % 4,
                                                  m * 128:(m + 1) * 128],
                                hT0[:, kk0, :],
                                start=(k0 == 0), stop=(k0 == n_kchunk - 1))
                        # interleave one deferred PE op from the previous
                        # block's postprocess; by now its inputs are ready
                        if deferred:
                            deferred.pop(0)()

                    for kq in range(4):
                        # one DMA brings 4 contraction chunks of hidden
                        if sb == 0 and kq in hT4_pre:
                            hT4 = hT4_pre[kq]
                        else:
                            hT4 = p1.tile([128, 4, 512], bf16, name="hT4",
                                          tag="hT4", bufs=3)
                            nc.sync.dma_start(
                                hT4, hr[:, 4 * kq:4 * kq + 4,
                                        sb * 512:(sb + 1) * 512])
                        if sb == 0 and kq == 0:
                            # consts needed from the first postprocess on;
                            # issued after the first wq/hidden loads
                            nc.sync.dma_start(pswap_sb, pswapc)
                            nc.sync.dma_start(ident_sb, identc)
                            nc.sync.dma_start(ones_sb, onesc)
                            nc.sync.dma_start(qkw_sb, qkw)
                        # rope-table chunks must be EMITTED before any rope
                        # op that reads them (emission order defines RAW vs
                        # WAR in Tile) -- chunks 0-2 land in sb0 kq1-3, the
                        # rest early in sb1 (first read is at sb6).
                        ci = None
                        if sb == 0 and 1 <= kq <= 3 and kq - 1 < n_cch:
                            ci = kq - 1
                        elif sb == 1 and kq + 3 < n_cch:
                            ci = kq + 3
                        if ci is not None:
                            nc.sync.dma_start(cos_chunks[ci],
                                              cosst[:, ci * csz:(ci + 1) * csz])
                            nc.sync.dma_start(sinn_chunks[ci],
                                              sinnst[:, ci * csz:(ci + 1) * csz])
                        for kk in range(4):
                            pend.append((kq * 4 + kk, hT4, kk))
                            if len(pend) >= 3:
                                flush_mm()
                    while pend:
                        flush_mm()

                    ssl = slice(sb * 512, (sb + 1) * 512)
                    # Free the psum banks fast: all copies + squares first.
                    # Everything downstream (stat matmuls, rope) is deferred
                    # into the next block's MM stream so PE never waits.
                    # The two q heads (m=0,1) share norm weight and score
                    # scale, so their stats run as one 1024-wide stream.
                    cpy01 = p1.tile([128, 2, 512], f32, name="cpy01",
                                    tag="cpy01", bufs=2)
                    nc.vector.tensor_copy(cpy01[:, 0, :], projps[0])
                    nc.vector.tensor_copy(cpy01[:, 1, :], projps[1])
                    cpy2 = p1.tile([128, 512], f32, name="cpy2", tag="cpy2",
                                   bufs=2)
                    nc.vector.tensor_copy(cpy2, projps[2])
                    # squares on GpSimd from the SBUF copies: keeps the psum
                    # free chain DVE-only and ACT out of the square work
                    sq01 = p1.tile([128, 2, 512], bf16, name="sq01",
                                   tag="sq01", bufs=2)
                    nc.gpsimd.tensor_mul(sq01, cpy01, cpy01)
                    sq2 = p1.tile([128, 512], bf16, name="sq2", tag="sq2",
                                  bufs=2)
                    nc.gpsimd.tensor_mul(sq2, cpy2, cpy2)
                    vT = p1.tile([128, 512], f32r, name="vT", tag="vT")
                    nc.vector.tensor_copy(vT, projps[3])

                    raws = {}

                    def emit_stats01(cpy01=cpy01, sq01=sq01):
                        ssps = p1ps.tile([128, 1024], f32, name="ssps",
                                         tag="ssps", bufs=1)
                        nc.tensor.matmul(ssps[:, 0:512], ones_sb,
                                         sq01[:, 0, :], start=True, stop=True)
                        nc.tensor.matmul(ssps[:, 512:1024], ones_sb,
                                         sq01[:, 1, :], start=True, stop=True)
                        tln = p1.tile([128, 1024], f32, name="tln", tag="tln")
                        nc.scalar.activation(
                            tln, ssps, mybir.ActivationFunctionType.Ln,
                            bias=qkw_sb[:, 2:3], scale=1.0 / 128.0)
                        rq = p1.tile([128, 1024], f32, name="rq", tag="rq")
                        # q heads fold the 1/sqrt(D) score scale in bias
                        nc.scalar.activation(
                            rq, tln, mybir.ActivationFunctionType.Exp,
                            bias=qkw_sb[:, 3:4], scale=-0.5)
                        raw01 = p1.tile([128, 2, 512], bf16, name="raw01",
                                        tag="raw01")
                        nc.vector.scalar_tensor_tensor(
                            raw01.rearrange("p a b -> p (a b)"),
                            cpy01.rearrange("p a b -> p (a b)"),
                            qkw_sb[:, 0:1], rq,
                            op0=mybir.AluOpType.mult,
                            op1=mybir.AluOpType.mult)
                        raws[0] = raw01[:, 0, :]
                        raws[1] = raw01[:, 1, :]

                    def emit_stats2(cpy2=cpy2, sq2=sq2):
                        # k head reuses half of the (bufs=1) wide psum tile
                        ssps = p1ps.tile([128, 1024], f32, name="ssps",
                                         tag="ssps", bufs=1)
                        nc.tensor.matmul(ssps[:, 0:512], ones_sb, sq2,
                                         start=True, stop=True)
                        tln = p1.tile([128, 512], f32, name="tln2", tag="tln2")
                        nc.scalar.activation(
                            tln, ssps[:, 0:512],
                            mybir.ActivationFunctionType.Ln,
                            bias=qkw_sb[:, 2:3], scale=1.0 / 128.0)
                        rq = p1.tile([128, 512], f32, name="rq2", tag="rq2")
                        nc.scalar.activation(
                            rq, tln, mybir.ActivationFunctionType.Exp,
                            scale=-0.5)
                        raw = p1.tile([128, 512], bf16, name="raw2",
                                      tag="raw2")
                        nc.vector.scalar_tensor_tensor(
                            raw, cpy2, qkw_sb[:, 1:2], rq,
                            op0=mybir.AluOpType.mult,
                            op1=mybir.AluOpType.mult)
                        raws[2] = raw

                    def make_rope(m, sb=sb):
                        def emit_rope():
                            raw = raws[m]
                            sslm = slice(sb * 512, (sb + 1) * 512)
                            # half-swap via PE permutation matmul
                            bsw = ptps.tile([128, 512], f32, name="bsw",
                                            tag="tps")
                            nc.tensor.matmul(bsw, pswap_sb, raw,
                                             start=True, stop=True)
                            ci, co = sb * 512 // csz, (sb * 512) % csz
                            # cos-mul on GpSimd (all-SBUF operands): runs in
                            # parallel with the DVE sin-mul, and keeps the
                            # phase-boundary DVE queue short
                            ttc = p1.tile([128, 512], bf16, name="ttc",
                                          tag="ttc")
                            nc.gpsimd.tensor_mul(
                                ttc, raw, cos_chunks[ci][:, co:co + 512])
                            tts = p1.tile([128, 512], bf16, name="tts",
                                          tag="tts")
                            nc.vector.tensor_mul(
                                tts, bsw, sinn_chunks[ci][:, co:co + 512])
                            nc.vector.tensor_add(qkT[:, m, sslm], ttc, tts)
                        return emit_rope

                    deferred.append(emit_stats01)
                    deferred.append(emit_stats2)
                    deferred.append(make_rope(0))
                    deferred.append(make_rope(1))
                    deferred.append(make_rope(2))

                    def emit_v(vT=vT, sb=sb):
                        vps = ptps.tile([128, 512], f32r, name="vps",
                                        tag="tps")
                        for j in range(4):
                            nc.tensor.transpose(
                                vps[:, j * 128:(j + 1) * 128],
                                vT[:, j * 128:(j + 1) * 128], ident_sb)
                        nc.vector.tensor_copy(
                            v_sb[:, 4 * sb:4 * sb + 4, :]
                            .rearrange("p a b -> p (a b)"),
                            vps)
                    deferred.append(emit_v)
                    if sb == n_sb - 1:
                        nc.sync.dma_start(
                            wo_sb, wo.rearrange("(h p) n -> p h n", p=128))
                def emit_warm(n):
                    # keep PE busy while the last block's stats/rope chains
                    # drain: a PE-idle window >3.4us here HAM-throttles the
                    # clock to 1.2GHz right as attention starts. Must be
                    # emitted BEFORE the waiting rope matmuls (PE is FIFO).
                    # Results are never read.
                    for _ in range(n):
                        wtile = ptps.tile([128, 512], f32, name="warm",
                                          tag="tps")
                        nc.tensor.matmul(wtile, ones_sb, qkT[:, 2, 0:512],
                                         start=True, stop=True)

                # final drain: the two stats closures launch their ACT
                # chains, then a warmup block covers the ~5us until raw01/
                # raw2 exist, then the rope/v closures drain.
                deferred.pop(0)()
                deferred.pop(0)()
                emit_warm(26)
                while deferred:
                    deferred.pop(0)()
                    emit_warm(2)

            # -------- Phases 2+3 interleaved: attention + output proj ------
            # Per (qb, h): kb pairs -> one [128,1024] psum, one wide exp into
            # a per-head slot buffer; PV matmuls accumulate per half.  The
            # softmax denominator: DVE adds the two halves of each pair
            # (contiguous reads), then an accumulating ones-matmul per pair
            # (144 total) does the partition reduction + broadcast in psum.
            # The denominator is ready at loop end, so each head's epilogue
            # follows its loop immediately; the output projection for qb-1
            # runs after both heads as pure filler PE work.
            with tc.tile_pool(name="p2s", bufs=2) as p2s, \
                 tc.tile_pool(name="oTp", bufs=4) as oTp, \
                 tc.tile_pool(name="p3", bufs=3) as p3, \
                 tc.tile_pool(name="ebp", bufs=1) as ebp, \
                 tc.tile_pool(name="scps_pool", bufs=2, space="PSUM") as scps_pool, \
                 tc.tile_pool(name="accps", bufs=1, space="PSUM") as accps, \
                 tc.tile_pool(name="mps", bufs=2, space="PSUM") as mps:
                ebufs = [
                    ebp.tile([128, n_qb * 2, 1024], bf16, name=f"ebuf{h}",
                             tag=f"ebuf{h}")
                    for h in range(HPC)
                ]
                cptog = [0]

                def attn_loop(qb, h, fillers):
                    qsl = slice(qb * 512, (qb + 1) * 512)
                    npair = 2 * qb + 2
                    ops = accps.tile([128, 512], f32, name="ops", tag="ops")
                    lps = accps.tile([128, 512], f32, name="lps", tag="lps")
                    esums = {}
                    for step in range(npair + 3):
                        if fillers:
                            fillers.pop(0)(True)
                        if step < npair:
                            p = step
                            kb0 = 2 * p
                            scps = scps_pool.tile([128, 1024], f32,
                                                  name="scps", tag="scps")
                            nc.tensor.matmul(
                                scps[:, 0:512],
                                qkT[:, 2, kb0 * 128:(kb0 + 1) * 128],
                                qkT[:, h, qsl], start=True, stop=True)
                            nc.tensor.matmul(
                                scps[:, 512:1024],
                                qkT[:, 2, (kb0 + 1) * 128:(kb0 + 2) * 128],
                                qkT[:, h, qsl], start=True, stop=True)
                            esb = ebufs[h][:, p, :]
                            nc.scalar.activation(
                                esb, scps, mybir.ActivationFunctionType.Exp)
                            if p >= 2 * qb:
                                # zero the k>q region of the diagonal pair
                                nc.gpsimd.affine_select(
                                    out=esb.rearrange("p (x q) -> p x q", x=2),
                                    in_=esb.rearrange("p (x q) -> p x q", x=2),
                                    compare_op=mybir.AluOpType.is_ge,
                                    fill=0.0,
                                    base=qb * 512 - kb0 * 128,
                                    pattern=[[-128, 2], [1, 512]],
                                    channel_multiplier=-1)
                        if step >= 1 and step - 1 < npair:
                            # pair-sum for the denominator (contiguous reads)
                            p = step - 1
                            esb = ebufs[h][:, p, :]
                            esum = p2s.tile([128, 512], bf16, name="esum",
                                            tag="esum", bufs=4)
                            nc.vector.tensor_add(esum, esb[:, 0:512],
                                                 esb[:, 512:1024])
                            esums[p] = esum
                        if step >= 3:
                            p = step - 3
                            kb0 = 2 * p
                            esb = ebufs[h][:, p, :]
                            nc.tensor.matmul(ops, v_sb[:, kb0, :],
                                             esb[:, 0:512],
                                             start=(p == 0), stop=False)
                            nc.tensor.matmul(ops, v_sb[:, kb0 + 1, :],
                                             esb[:, 512:1024],
                                             start=False, stop=(p == npair - 1))
                            nc.tensor.matmul(lps, ones_sb, esums.pop(p),
                                             start=(p == 0),
                                             stop=(p == npair - 1))
                    return ops, lps

                def emit_lfinish(ops, lps):
                    tl2 = p2s.tile([128, 512], f32, name="tl2", tag="tl2")
                    nc.scalar.activation(tl2, lps,
                                         mybir.ActivationFunctionType.Ln)
                    rl = p2s.tile([128, 512], f32, name="rl", tag="rl")
                    nc.scalar.activation(rl, tl2,
                                         mybir.ActivationFunctionType.Exp,
                                         scale=-1.0)
                    ot = oTp.tile([128, 512], bf16, name="ot", tag="ot")
                    nc.vector.tensor_mul(ot, ops, rl)
                    return ot

                def make_wo_units(qb, oTt):
                    # 16 single-(st4, nb) closures, interleaved one-per-step
                    # into the NEXT q-block's attention loops as PE filler
                    stg_state = {}

                    def make_unit(st4, nb):
                        def emit(in_loop):
                            st = qb * 4 + st4
                            stsl = slice(st * 128, (st + 1) * 128)
                            s4 = slice(st4 * 128, (st4 + 1) * 128)
                            if nb == 0:
                                stg_state[st4] = p3.tile(
                                    [128, n_nb, 512], bf16, name="stg4",
                                    tag="stg4")
                            stg4 = stg_state[st4]
                            nbsl = slice(nb * 512, (nb + 1) * 512)
                            if in_loop:
                                wops = mps.tile([128, 512], f32, name="wops",
                                                tag="mps")
                            else:
                                # flushed units rotate over 4 psum banks
                                # (mps x2 + the attention accumulators, idle
                                # during a flush) so stage copies overlap MMs
                                wtag = ("mps", "ops", "mps", "lps")[
                                    cptog[0] % 4]
                                wpool = mps if wtag == "mps" else accps
                                wops = wpool.tile([128, 512], f32,
                                                  name="wops", tag=wtag)
                            for h in range(HPC):
                                nc.tensor.matmul(wops, oTt[h][:, s4],
                                                 wo_sb[:, h, nbsl],
                                                 start=(h == 0),
                                                 stop=(h == HPC - 1))
                            # in-loop stages go to DVE (ACT paces the exp
                            # stream); flushed stages alternate ACT/DVE
                            if in_loop or cptog[0] % 2 == 0:
                                nc.vector.tensor_copy(stg4[:, nb, :], wops)
                            else:
                                nc.scalar.copy(stg4[:, nb, :], wops)
                            cptog[0] += 1
                            if nb == n_nb - 1:
                                nc.sync.dma_start(
                                    out[stsl, :],
                                    stg4.rearrange("p a b -> p (a b)"))
                        return emit

                    return [make_unit(st4, nb)
                            for st4 in range(4) for nb in range(n_nb)]

                # attention q-blocks in an order whose first block only needs
                # early-sequence K/V: the last seq block's rope chain then
                # overlaps the first attention loop instead of stalling PE
                fillers = []
                for qb in [2, 3, 4, 5, 6, 7, 0, 1]:
                    ops0, lps0 = attn_loop(qb, 0, fillers)
                    ot0 = emit_lfinish(ops0, lps0)
                    ops1, lps1 = attn_loop(qb, 1, fillers)
                    ot1 = emit_lfinish(ops1, lps1)
                    while fillers:
                        fillers.pop(0)(False)
                    fillers = make_wo_units(qb, [ot0, ot1])
                while fillers:
                    fillers.pop(0)(False)

    nc.compile()
    return nc


def _host_inputs(hidden_state, Wq, Wk, Wv, Wo, q_norm_w, k_norm_w, position_ids,
                 s_len):
    """Build the 8 per-core input maps."""
    import ml_dtypes
    bf16 = ml_dtypes.bfloat16

    half = D // 2
    pos = np.asarray(position_ids).astype(np.float64)
    inv_freq = 1.0 / (THETA ** (np.arange(half, dtype=np.float64) / half))
    ang = pos[:, None] * inv_freq[None, :]          # [S, half]
    cosT = np.cos(ang).T.astype(np.float32)         # [half, S]
    sinT = np.sin(ang).T.astype(np.float32)
    cosst = np.concatenate([cosT, cosT], axis=0).astype(bf16)       # [128, S]
    sinnst = np.concatenate([-sinT, sinT], axis=0).astype(bf16)     # [128, S]
    ident = np.eye(128, dtype=np.float32)
    ones = np.ones((128, 128), dtype=bf16)
    pswap = np.roll(np.eye(128), 64, axis=0).astype(bf16)
    hiddenT = np.asarray(hidden_state, dtype=np.float32).T.astype(bf16)
    qw = np.asarray(q_norm_w, dtype=np.float32)
    kw = np.asarray(k_norm_w, dtype=np.float32)
    epsc = np.full(D, EPS, dtype=np.float32)
    nbq = np.full(D, -0.5 * np.log(128.0), dtype=np.float32)
    qkw = np.stack([qw, kw, epsc, nbq], axis=1)     # [D, 4]

    in_maps = []
    for c in range(NCORES):
        wq_sl = Wq[:, c * HPC * D:(c + 1) * HPC * D]
        wk_sl = Wk[:, c * D:(c + 1) * D]
        wv_sl = Wv[:, c * D:(c + 1) * D]
        wqkv = np.concatenate([wq_sl, wk_sl, wv_sl], axis=1).astype(bf16)
        wo_sl = np.ascontiguousarray(
            Wo[c * HPC * D:(c + 1) * HPC * D, :]).astype(bf16)
        in_maps.append({
            "hiddenT": hiddenT,
            "wqkv": wqkv,
            "wo": wo_sl,
            "qkw": qkw,
            "cosst": cosst,
            "sinnst": sinnst,
            "identc": ident,
            "onesc": ones,
            "pswapc": pswap,
        })
    return in_maps


def kernel(hidden_state, Wq, Wk, Wv, Wo, q_norm_w, k_norm_w, position_ids,
           _s_len=None, _trace=False, **_ignored):
    from concourse.bass_utils import run_bass_kernel_spmd

    # accept jax or numpy inputs
    hidden_state = np.asarray(hidden_state)
    Wq, Wk, Wv, Wo = (np.asarray(w) for w in (Wq, Wk, Wv, Wo))
    q_norm_w = np.asarray(q_norm_w)
    k_norm_w = np.asarray(k_norm_w)
    position_ids = np.asarray(position_ids)

    s_len = int(hidden_state.shape[0]) if _s_len is None else _s_len
    if s_len not in _CACHE:
        _CACHE[s_len] = _build(s_len)
    nc = _CACHE[s_len]

    in_maps = _host_inputs(hidden_state, Wq, Wk, Wv, Wo, q_norm_w, k_norm_w,
                           position_ids, s_len)
    res = run_bass_kernel_spmd(nc, in_maps, core_ids=list(range(NCORES)),
                               trace=_trace)
    kernel._last = res
    acc = res.results[0]["out"].astype(np.float32)
    for c in range(1, NCORES):
        acc += res.results[c]["out"].astype(np.float32)
    return acc
